# revision 3
# baseline (speedup 1.0000x reference)
"""AnyUp sparse-attention upsampler on 8 Trainium2 NeuronCores.

Sharding: the full-res (256x256) encoder/query/key branches are sharded
row-wise into 8 stripes of 32 rows (pure spatial data parallel). GroupNorm
needs global spatial stats -> lax.psum of per-stripe partial sums. The
low-res (32x32) branch is cheap and computed replicated on every core;
the pooled key image (computed from sharded full-res rows) is all_gathered
so every core holds the full 32x32 key/value maps. Windowed cross-attention
is then computed locally per stripe (each stripe's 4 key-cell rows read a
+-2 row halo from the replicated key map). Output stripes are gathered on
host into the full [2,384,256,256] tensor.
"""

import numpy as np
import jax
import jax.numpy as jnp
from jax import lax
from functools import partial

QK = 128
HEADS = 4
GROUPS = 8
EPS = 1e-5
B = 2
CV = 384
H = W = 256
HF = WF = 32
NC = 8            # cores
HL = H // NC      # 32 local query rows per core
KL = HF // NC     # 4 local key-cell rows per core
R = H // HF       # 8 upsampling ratio
DH = QK // HEADS  # 32
DV = CV // HEADS  # 96
HALF = 2          # max(1, round(0.1*32/2))
WIN = 2 * HALF + 1


def _conv1x1(x, w):
    return jnp.einsum('bchw,oc->bohw', x, w)


def _gn_global(x, gamma, beta):
    # x: [B,C,HL,W] stripe; stats over the full H x W map via psum.
    b, c, h, w = x.shape
    xg = x.reshape(b, GROUPS, c // GROUPS, h, w)
    s1 = lax.psum(xg.sum(axis=(2, 3, 4)), 'i')
    s2 = lax.psum((xg * xg).sum(axis=(2, 3, 4)), 'i')
    n = (c // GROUPS) * H * w
    m = s1 / n
    v = s2 / n - m * m
    xn = (xg - m[..., None, None, None]) * lax.rsqrt(v[..., None, None, None] + EPS)
    xn = xn.reshape(b, c, h, w)
    return xn * gamma[None, :, None, None] + beta[None, :, None, None]


def _gn_local(x, gamma, beta):
    # full map held locally (low-res branch) — matches reference exactly
    b, c, h, w = x.shape
    xg = x.reshape(b, GROUPS, c // GROUPS, h, w)
    m = xg.mean(axis=(2, 3, 4), keepdims=True)
    v = xg.var(axis=(2, 3, 4), keepdims=True)
    xn = ((xg - m) / jnp.sqrt(v + EPS)).reshape(b, c, h, w)
    return xn * gamma[None, :, None, None] + beta[None, :, None, None]


def _conv_reflect(x, w):
    # k x k conv with reflect padding, as shifted-slice einsums
    # (conv_general_dilated ICEs this neuronxcc build)
    k = w.shape[-1]
    p = k // 2
    h, ww = x.shape[-2:]
    xp = jnp.pad(x, ((0, 0), (0, 0), (p, p), (p, p)), mode='reflect')
    out = None
    for dy in range(k):
        for dx in range(k):
            t = jnp.einsum('bchw,oc->bohw', xp[:, :, dy:dy + h, dx:dx + ww], w[:, :, dy, dx])
            out = t if out is None else out + t
    return out


def _rope_stripe(e, row0):
    # e: [B,C,HL,W]; axial 2D RoPE with absolute row coordinates row0..row0+HL
    b, c, h, w = e.shape
    d4 = c // 4
    freqs = 100.0 ** (jnp.arange(d4, dtype=jnp.float32) / d4)
    ys = (row0 + jnp.arange(h, dtype=jnp.float32) + 0.5) / H
    xs = (jnp.arange(w, dtype=jnp.float32) + 0.5) / W
    ay = ys[:, None] * freqs[None, :] * jnp.pi
    ax = xs[:, None] * freqs[None, :] * jnp.pi
    eh = e.transpose(0, 2, 3, 1)
    vy = eh[..., :c // 2].reshape(b, h, w, d4, 2)
    vx = eh[..., c // 2:].reshape(b, h, w, d4, 2)

    def rot(v, cc, ss):
        return jnp.stack([v[..., 0] * cc - v[..., 1] * ss,
                          v[..., 0] * ss + v[..., 1] * cc], axis=-1)

    vy = rot(vy, jnp.cos(ay)[None, :, None, :], jnp.sin(ay)[None, :, None, :])
    vx = rot(vx, jnp.cos(ax)[None, None, :, :], jnp.sin(ax)[None, None, :, :])
    out = jnp.concatenate([vy.reshape(b, h, w, c // 2),
                           vx.reshape(b, h, w, c // 2)], axis=-1)
    return out.transpose(0, 3, 1, 2)


def _fwd_stripe(image_s, features, pre_img, pre_key, pre_query, lfu_w, pre_agg,
                rb_w1, rb_w2, rb_gamma, rb_beta):
    idx = lax.axis_index('i')
    row0 = (idx * HL).astype(jnp.float32)

    def rb_g(x, i):
        h = _conv1x1(x, rb_w1[i])
        h = _gn_global(h, rb_gamma[i], rb_beta[i])
        h = jax.nn.silu(h)
        h = _conv1x1(h, rb_w2[i])
        return x + h

    def rb_l(x, i):
        h = _conv1x1(x, rb_w1[i])
        h = _gn_local(h, rb_gamma[i], rb_beta[i])
        h = jax.nn.silu(h)
        h = _conv1x1(h, rb_w2[i])
        return x + h

    # ---- full-res trunk (sharded by rows) ----
    enc = rb_g(rb_g(_conv1x1(image_s, pre_img), 0), 1)
    enc = _rope_stripe(enc, row0)

    q = rb_g(rb_g(_conv1x1(enc, pre_query), 4), 5)            # [B,QK,HL,W]
    k_pre = rb_g(rb_g(_conv1x1(enc, pre_key), 2), 3)          # [B,QK,HL,W]
    k_img_loc = k_pre.reshape(B, QK, KL, R, WF, R).mean(axis=(3, 5))  # [B,QK,KL,WF]

    # ---- low-res branch (replicated) ----
    fn = features / jnp.maximum(jnp.linalg.norm(features, axis=1, keepdims=True), 1e-12)
    kfe = _conv_reflect(fn.mean(axis=1, keepdims=True), lfu_w)
    kfe = rb_l(rb_l(kfe, 6), 7)                                # [B,QK,HF,WF]

    # gather the pooled key image across cores -> full 32x32 map
    k_img = lax.all_gather(k_img_loc, 'i')                     # [NC,B,QK,KL,WF]
    k_img = k_img.transpose(1, 2, 0, 3, 4).reshape(B, QK, HF, WF)

    kk = jnp.concatenate([k_img, kfe], axis=1)
    kk = rb_l(rb_l(_conv_reflect(kk, pre_agg), 8), 9)          # [B,QK,HF,WF]

    # ---- windowed cross-attention for the local stripe (gather-free) ----
    kr0 = idx * KL
    qb = q.reshape(B, HEADS, DH, KL, R, WF, R)                 # b n d i u j v

    # zero-pad key/value maps by HALF on each spatial side; invalid window
    # positions are masked to -1e9 before softmax (-> exactly 0 weight in f32)
    kpad = jnp.pad(kk, ((0, 0), (0, 0), (HALF, HALF), (HALF, HALF)))
    vpad = jnp.pad(features, ((0, 0), (0, 0), (HALF, HALF), (HALF, HALF)))
    # local halo window: rows kr0-HALF .. kr0+KL+HALF in padded coords = kr0..
    khalo = lax.dynamic_slice(kpad, (0, 0, kr0, 0), (B, QK, KL + 2 * HALF, WF + 2 * HALF))
    vhalo = lax.dynamic_slice(vpad, (0, 0, kr0, 0), (B, CV, KL + 2 * HALF, WF + 2 * HALF))
    khalo = khalo.reshape(B, HEADS, DH, KL + 2 * HALF, WF + 2 * HALF)
    vhalo = vhalo.reshape(B, HEADS, DV, KL + 2 * HALF, WF + 2 * HALF)

    il = kr0 + jnp.arange(KL)                                  # absolute key rows
    jl = jnp.arange(WF)
    dy = jnp.arange(-HALF, HALF + 1)
    rvalid = (il[:, None] + dy[None, :] >= 0) & (il[:, None] + dy[None, :] < HF)
    cvalid = (jl[:, None] + dy[None, :] >= 0) & (jl[:, None] + dy[None, :] < WF)
    # mask[i, j, p] with p = oy*WIN+ox
    wmask = (rvalid[:, None, :, None] & cvalid[None, :, None, :]).reshape(KL, WF, WIN * WIN)

    ksh = jnp.stack([khalo[:, :, :, oy:oy + KL, ox:ox + WF]
                     for oy in range(WIN) for ox in range(WIN)], axis=0)  # [P,b,n,d,i,j]
    vsh = jnp.stack([vhalo[:, :, :, oy:oy + KL, ox:ox + WF]
                     for oy in range(WIN) for ox in range(WIN)], axis=0)  # [P,b,n,e,i,j]

    scale = 1.0 / np.sqrt(DH)
    scores = jnp.einsum('bndiujv,pbndij->bniujvp', qb, ksh) * scale
    scores = jnp.where(wmask[None, None, :, None, :, None, :], scores, -1e9)
    attn = jax.nn.softmax(scores, axis=-1)
    out = jnp.einsum('bniujvp,pbneij->bneiujv', attn, vsh)     # [B,N,DV,KL,R,WF,R]

    out = out.reshape(B, CV, KL, R, WF, R).reshape(B, CV, HL, W)
    return out


_fwd = None


def _get_fwd():
    global _fwd
    if _fwd is None:
        _fwd = jax.pmap(
            _fwd_stripe, axis_name='i',
            in_axes=(0,) + (None,) * 10,
            out_axes=0,
        )
    return _fwd


def kernel(**inputs):
    image = np.ascontiguousarray(np.asarray(inputs['image'], dtype=np.float32))
    img_s = image.reshape(B, 3, NC, HL, W).transpose(2, 0, 1, 3, 4)
    args = [np.asarray(inputs[k], dtype=np.float32) for k in
            ('features', 'pre_img', 'pre_key', 'pre_query', 'lfu_w', 'pre_agg',
             'rb_w1', 'rb_w2', 'rb_gamma', 'rb_beta')]
    out = _get_fwd()(img_s, *args)                      # [NC,B,CV,HL,W]
    out = np.asarray(out).transpose(1, 2, 0, 3, 4).reshape(B, CV, H, W)
    return out


# revision 9
# speedup vs baseline: 1.0862x; 1.0862x over previous
"""AnyUp sparse-attention upsampler on 8 Trainium2 NeuronCores.

Sharding: the full-res (256x256) encoder/query/key branches are sharded
row-wise into 8 stripes of 32 rows (pure spatial data parallel). GroupNorm
needs global spatial stats -> lax.psum of per-stripe partial sums. The
low-res (32x32) branch is cheap and computed replicated on every core;
the pooled key image (computed from sharded full-res rows) is all_gathered
so every core holds the full 32x32 key/value maps. Windowed cross-attention
is then computed locally per stripe (each stripe's 4 key-cell rows read a
+-2 row halo from the replicated key map). Output stripes are gathered on
host into the full [2,384,256,256] tensor.
"""

import numpy as np
import jax
import jax.numpy as jnp
from jax import lax
from functools import partial

QK = 128
HEADS = 4
GROUPS = 8
EPS = 1e-5
B = 2
CV = 384
H = W = 256
HF = WF = 32
NC = 8            # cores
HL = H // NC      # 32 local query rows per core
KL = HF // NC     # 4 local key-cell rows per core
R = H // HF       # 8 upsampling ratio
DH = QK // HEADS  # 32
DV = CV // HEADS  # 96
HALF = 2          # max(1, round(0.1*32/2))
WIN = 2 * HALF + 1


def _conv1x1(x, w):
    return jnp.einsum('bchw,oc->bohw', x, w)


BF = jnp.bfloat16
F32 = jnp.float32


def _gn_apply(x, gamma, beta, m, v):
    # folded affine: out = x*a + b with a=[B,C], b=[B,C]
    b_, c, h, w = x.shape
    cg = c // GROUPS
    inv = lax.rsqrt(v + EPS)                                   # [B,G] f32
    a = jnp.repeat(inv, cg, axis=1) * gamma[None, :]           # [B,C]
    bb = beta[None, :] - jnp.repeat(m * inv, cg, axis=1) * gamma[None, :]
    return x * a.astype(x.dtype)[:, :, None, None] + bb.astype(x.dtype)[:, :, None, None]


def _gn_global(x, gamma, beta):
    # x: [B,C,HL,W] stripe (bf16); stats in f32 over the full map via psum.
    b, c, h, w = x.shape
    xg = x.reshape(b, GROUPS, c // GROUPS, h, w)
    s1 = lax.psum(jnp.sum(xg, axis=(2, 3, 4), dtype=F32), 'i')
    s2 = lax.psum(jnp.sum(xg * xg, axis=(2, 3, 4), dtype=F32), 'i')
    n = (c // GROUPS) * H * w
    m = s1 / n
    v = s2 / n - m * m
    return _gn_apply(x, gamma, beta, m, v)


def _gn_local(x, gamma, beta):
    # full map held locally (low-res branch)
    b, c, h, w = x.shape
    xg = x.reshape(b, GROUPS, c // GROUPS, h, w)
    s1 = jnp.sum(xg, axis=(2, 3, 4), dtype=F32)
    s2 = jnp.sum(xg * xg, axis=(2, 3, 4), dtype=F32)
    n = (c // GROUPS) * h * w
    m = s1 / n
    v = s2 / n - m * m
    return _gn_apply(x, gamma, beta, m, v)


def _conv_reflect(x, w):
    # k x k conv with reflect padding, as shifted-slice einsums
    # (conv_general_dilated ICEs this neuronxcc build)
    k = w.shape[-1]
    p = k // 2
    h, ww = x.shape[-2:]
    xp = jnp.pad(x, ((0, 0), (0, 0), (p, p), (p, p)), mode='reflect')
    out = None
    for dy in range(k):
        for dx in range(k):
            t = jnp.einsum('bchw,oc->bohw', xp[:, :, dy:dy + h, dx:dx + ww], w[:, :, dy, dx])
            out = t if out is None else out + t
    return out


def _rope_stripe(e, row0):
    # e: [B,C,HL,W]; axial 2D RoPE with absolute row coordinates row0..row0+HL
    b, c, h, w = e.shape
    d4 = c // 4
    freqs = 100.0 ** (jnp.arange(d4, dtype=jnp.float32) / d4)
    ys = (row0 + jnp.arange(h, dtype=jnp.float32) + 0.5) / H
    xs = (jnp.arange(w, dtype=jnp.float32) + 0.5) / W
    ay = ys[:, None] * freqs[None, :] * jnp.pi
    ax = xs[:, None] * freqs[None, :] * jnp.pi
    eh = e.transpose(0, 2, 3, 1)
    vy = eh[..., :c // 2].reshape(b, h, w, d4, 2)
    vx = eh[..., c // 2:].reshape(b, h, w, d4, 2)

    def rot(v, cc, ss):
        cc = cc.astype(v.dtype)
        ss = ss.astype(v.dtype)
        return jnp.stack([v[..., 0] * cc - v[..., 1] * ss,
                          v[..., 0] * ss + v[..., 1] * cc], axis=-1)

    vy = rot(vy, jnp.cos(ay)[None, :, None, :], jnp.sin(ay)[None, :, None, :])
    vx = rot(vx, jnp.cos(ax)[None, None, :, :], jnp.sin(ax)[None, None, :, :])
    out = jnp.concatenate([vy.reshape(b, h, w, c // 2),
                           vx.reshape(b, h, w, c // 2)], axis=-1)
    return out.transpose(0, 3, 1, 2)


def _fwd_stripe(image_s, features, pre_img, pre_key, pre_query, lfu_w, pre_agg,
                rb_w1, rb_w2, rb_gamma, rb_beta):
    idx = lax.axis_index('i')
    row0 = (idx * HL).astype(jnp.float32)

    # bf16 compute everywhere except GN stats / softmax accumulation
    image_s = image_s.astype(BF)
    pre_img = pre_img.astype(BF)
    pre_key = pre_key.astype(BF)
    pre_query = pre_query.astype(BF)
    lfu_w = lfu_w.astype(BF)
    pre_agg = pre_agg.astype(BF)
    rb_w1 = rb_w1.astype(BF)
    rb_w2 = rb_w2.astype(BF)

    def rb_g(x, i):
        h = _conv1x1(x, rb_w1[i])
        h = _gn_global(h, rb_gamma[i], rb_beta[i])
        h = jax.nn.silu(h)
        h = _conv1x1(h, rb_w2[i])
        return x + h

    def rb_l(x, i):
        h = _conv1x1(x, rb_w1[i])
        h = _gn_local(h, rb_gamma[i], rb_beta[i])
        h = jax.nn.silu(h)
        h = _conv1x1(h, rb_w2[i])
        return x + h

    # ---- full-res trunk (sharded by rows) ----
    enc = rb_g(rb_g(_conv1x1(image_s, pre_img), 0), 1)
    enc = _rope_stripe(enc, row0)

    q = rb_g(rb_g(_conv1x1(enc, pre_query), 4), 5)            # [B,QK,HL,W]
    k_pre = rb_g(rb_g(_conv1x1(enc, pre_key), 2), 3)          # [B,QK,HL,W]
    k_img_loc = k_pre.reshape(B, QK, KL, R, WF, R).mean(axis=(3, 5))  # [B,QK,KL,WF]

    # ---- low-res branch (replicated); norm in f32, rest bf16 ----
    fn = features / jnp.maximum(jnp.linalg.norm(features, axis=1, keepdims=True), 1e-12)
    kfe = _conv_reflect(fn.mean(axis=1, keepdims=True).astype(BF), lfu_w)
    kfe = rb_l(rb_l(kfe, 6), 7)                                # [B,QK,HF,WF]

    # gather the pooled key image across cores -> full 32x32 map
    k_img = lax.all_gather(k_img_loc, 'i')                     # [NC,B,QK,KL,WF]
    k_img = k_img.transpose(1, 2, 0, 3, 4).reshape(B, QK, HF, WF)

    kk = jnp.concatenate([k_img, kfe], axis=1)
    kk = rb_l(rb_l(_conv_reflect(kk, pre_agg), 8), 9)          # [B,QK,HF,WF]

    # ---- windowed cross-attention for the local stripe (gather-free) ----
    kr0 = idx * KL
    qb = q.reshape(B, HEADS, DH, KL, R, WF, R)                 # b n d i u j v

    # zero-pad key/value maps by HALF on each spatial side; invalid window
    # positions are masked to -1e9 before softmax (-> exactly 0 weight in f32)
    kpad = jnp.pad(kk, ((0, 0), (0, 0), (HALF, HALF), (HALF, HALF)))
    vpad = jnp.pad(features.astype(BF), ((0, 0), (0, 0), (HALF, HALF), (HALF, HALF)))
    # local halo window: rows kr0-HALF .. kr0+KL+HALF in padded coords = kr0..
    khalo = lax.dynamic_slice(kpad, (0, 0, kr0, 0), (B, QK, KL + 2 * HALF, WF + 2 * HALF))
    vhalo = lax.dynamic_slice(vpad, (0, 0, kr0, 0), (B, CV, KL + 2 * HALF, WF + 2 * HALF))
    khalo = khalo.reshape(B, HEADS, DH, KL + 2 * HALF, WF + 2 * HALF)
    vhalo = vhalo.reshape(B, HEADS, DV, KL + 2 * HALF, WF + 2 * HALF)

    il = kr0 + jnp.arange(KL)                                  # absolute key rows
    jl = jnp.arange(WF)
    dy = jnp.arange(-HALF, HALF + 1)
    rvalid = (il[:, None] + dy[None, :] >= 0) & (il[:, None] + dy[None, :] < HF)
    cvalid = (jl[:, None] + dy[None, :] >= 0) & (jl[:, None] + dy[None, :] < WF)
    # mask[i, j, p] with p = oy*WIN+ox
    wmask = (rvalid[:, None, :, None] & cvalid[None, :, None, :]).reshape(KL, WF, WIN * WIN)

    ksh = jnp.stack([khalo[:, :, :, oy:oy + KL, ox:ox + WF]
                     for oy in range(WIN) for ox in range(WIN)], axis=0)  # [P,b,n,d,i,j]
    vsh = jnp.stack([vhalo[:, :, :, oy:oy + KL, ox:ox + WF]
                     for oy in range(WIN) for ox in range(WIN)], axis=0)  # [P,b,n,e,i,j]

    scale = np.float32(1.0 / np.sqrt(DH))
    scores = jnp.einsum('bndiujv,pbndij->bniujvp', qb, ksh,
                        preferred_element_type=F32) * scale
    scores = jnp.where(wmask[None, None, :, None, :, None, :], scores, -1e9)
    m = scores.max(axis=-1, keepdims=True)
    e = jnp.exp(scores - m).astype(BF)
    ssum = jnp.sum(e, axis=-1, keepdims=True, dtype=F32)
    attn = e * (1.0 / ssum).astype(BF)
    out = jnp.einsum('bniujvp,pbneij->bneiujv', attn, vsh,
                     preferred_element_type=F32)               # [B,N,DV,KL,R,WF,R]

    out = out.reshape(B, CV, KL, R, WF, R).reshape(B, CV, HL, W)
    return out.astype(F32)


_fwd = None


def _get_fwd():
    global _fwd
    if _fwd is None:
        _fwd = jax.pmap(
            _fwd_stripe, axis_name='i',
            in_axes=(0,) + (None,) * 10,
            out_axes=0,
        )
    return _fwd


def kernel(**inputs):
    image = np.ascontiguousarray(np.asarray(inputs['image'], dtype=np.float32))
    img_s = image.reshape(B, 3, NC, HL, W).transpose(2, 0, 1, 3, 4)
    args = [np.asarray(inputs[k], dtype=np.float32) for k in
            ('features', 'pre_img', 'pre_key', 'pre_query', 'lfu_w', 'pre_agg',
             'rb_w1', 'rb_w2', 'rb_gamma', 'rb_beta')]
    out = _get_fwd()(img_s, *args)                      # [NC,B,CV,HL,W]
    out = np.asarray(out).transpose(1, 2, 0, 3, 4).reshape(B, CV, H, W)
    return out


# revision 12
# speedup vs baseline: 179.1250x; 164.9143x over previous
"""AnyUp sparse-attention upsampler on 8 Trainium2 NeuronCores.

Sharding: the full-res (256x256) encoder/query/key branches are sharded
row-wise into 8 stripes of 32 rows (pure spatial data parallel). GroupNorm
needs global spatial stats -> lax.psum of per-stripe partial sums. The
low-res (32x32) branch is cheap and computed replicated on every core;
the pooled key image (computed from sharded full-res rows) is all_gathered
so every core holds the full 32x32 key/value maps. Windowed cross-attention
is then computed locally per stripe (each stripe's 4 key-cell rows read a
+-2 row halo from the replicated key map). Output stripes are gathered on
host into the full [2,384,256,256] tensor.
"""

import numpy as np
import jax
import jax.numpy as jnp
from jax import lax
from functools import partial

QK = 128
HEADS = 4
GROUPS = 8
EPS = 1e-5
B = 2
CV = 384
H = W = 256
HF = WF = 32
NC = 8            # cores
HL = H // NC      # 32 local query rows per core
KL = HF // NC     # 4 local key-cell rows per core
R = H // HF       # 8 upsampling ratio
DH = QK // HEADS  # 32
DV = CV // HEADS  # 96
HALF = 2          # max(1, round(0.1*32/2))
WIN = 2 * HALF + 1


def _conv1x1(x, w):
    return jnp.einsum('bchw,oc->bohw', x, w)


def _gn_apply(x, gamma, beta, m, v):
    # folded per-channel affine: out = x*a + b, a/b: [B,C] broadcast over h,w
    cg = x.shape[1] // GROUPS
    inv = lax.rsqrt(v + EPS)                                   # [B,G]
    a = jnp.repeat(inv, cg, axis=1) * gamma[None, :]
    bb = beta[None, :] - jnp.repeat(m * inv, cg, axis=1) * gamma[None, :]
    return x * a[:, :, None, None] + bb[:, :, None, None]


def _gn_global(x, gamma, beta):
    # x: [B,C,HL,W] stripe; stats over the full H x W map via psum.
    b, c, h, w = x.shape
    xg = x.reshape(b, GROUPS, c // GROUPS, h, w)
    s1 = lax.psum(xg.sum(axis=(2, 3, 4)), 'i')
    s2 = lax.psum((xg * xg).sum(axis=(2, 3, 4)), 'i')
    n = (c // GROUPS) * H * w
    m = s1 / n
    v = s2 / n - m * m
    return _gn_apply(x, gamma, beta, m, v)


def _gn_local(x, gamma, beta):
    # full map held locally (low-res branch)
    b, c, h, w = x.shape
    xg = x.reshape(b, GROUPS, c // GROUPS, h, w)
    s1 = xg.sum(axis=(2, 3, 4))
    s2 = (xg * xg).sum(axis=(2, 3, 4))
    n = (c // GROUPS) * h * w
    m = s1 / n
    v = s2 / n - m * m
    return _gn_apply(x, gamma, beta, m, v)


def _conv_reflect(x, w):
    # k x k conv with reflect padding, as shifted-slice einsums
    # (conv_general_dilated ICEs this neuronxcc build)
    k = w.shape[-1]
    p = k // 2
    h, ww = x.shape[-2:]
    xp = jnp.pad(x, ((0, 0), (0, 0), (p, p), (p, p)), mode='reflect')
    out = None
    for dy in range(k):
        for dx in range(k):
            t = jnp.einsum('bchw,oc->bohw', xp[:, :, dy:dy + h, dx:dx + ww], w[:, :, dy, dx])
            out = t if out is None else out + t
    return out


def _rope_stripe(e, row0):
    # e: [B,C,HL,W]; axial 2D RoPE with absolute row coordinates row0..row0+HL
    b, c, h, w = e.shape
    d4 = c // 4
    freqs = 100.0 ** (jnp.arange(d4, dtype=jnp.float32) / d4)
    ys = (row0 + jnp.arange(h, dtype=jnp.float32) + 0.5) / H
    xs = (jnp.arange(w, dtype=jnp.float32) + 0.5) / W
    ay = ys[:, None] * freqs[None, :] * jnp.pi
    ax = xs[:, None] * freqs[None, :] * jnp.pi
    eh = e.transpose(0, 2, 3, 1)
    vy = eh[..., :c // 2].reshape(b, h, w, d4, 2)
    vx = eh[..., c // 2:].reshape(b, h, w, d4, 2)

    def rot(v, cc, ss):
        return jnp.stack([v[..., 0] * cc - v[..., 1] * ss,
                          v[..., 0] * ss + v[..., 1] * cc], axis=-1)

    vy = rot(vy, jnp.cos(ay)[None, :, None, :], jnp.sin(ay)[None, :, None, :])
    vx = rot(vx, jnp.cos(ax)[None, None, :, :], jnp.sin(ax)[None, None, :, :])
    out = jnp.concatenate([vy.reshape(b, h, w, c // 2),
                           vx.reshape(b, h, w, c // 2)], axis=-1)
    return out.transpose(0, 3, 1, 2)


def _fwd_stripe(image_s, features, pre_img, pre_key, pre_query, lfu_w, pre_agg,
                rb_w1, rb_w2, rb_gamma, rb_beta):
    idx = lax.axis_index('i')
    row0 = (idx * HL).astype(jnp.float32)

    def rb_g(x, i):
        h = _conv1x1(x, rb_w1[i])
        h = _gn_global(h, rb_gamma[i], rb_beta[i])
        h = jax.nn.silu(h)
        h = _conv1x1(h, rb_w2[i])
        return x + h

    def rb_l(x, i):
        h = _conv1x1(x, rb_w1[i])
        h = _gn_local(h, rb_gamma[i], rb_beta[i])
        h = jax.nn.silu(h)
        h = _conv1x1(h, rb_w2[i])
        return x + h

    # ---- full-res trunk (sharded by rows) ----
    enc = rb_g(rb_g(_conv1x1(image_s, pre_img), 0), 1)
    enc = _rope_stripe(enc, row0)

    q = rb_g(rb_g(_conv1x1(enc, pre_query), 4), 5)            # [B,QK,HL,W]
    k_pre = rb_g(rb_g(_conv1x1(enc, pre_key), 2), 3)          # [B,QK,HL,W]
    k_img_loc = k_pre.reshape(B, QK, KL, R, WF, R).mean(axis=(3, 5))  # [B,QK,KL,WF]

    # ---- low-res branch (replicated) ----
    fn = features / jnp.maximum(jnp.linalg.norm(features, axis=1, keepdims=True), 1e-12)
    kfe = _conv_reflect(fn.mean(axis=1, keepdims=True), lfu_w)
    kfe = rb_l(rb_l(kfe, 6), 7)                                # [B,QK,HF,WF]

    # gather the pooled key image across cores -> full 32x32 map
    k_img = lax.all_gather(k_img_loc, 'i')                     # [NC,B,QK,KL,WF]
    k_img = k_img.transpose(1, 2, 0, 3, 4).reshape(B, QK, HF, WF)

    kk = jnp.concatenate([k_img, kfe], axis=1)
    kk = rb_l(rb_l(_conv_reflect(kk, pre_agg), 8), 9)          # [B,QK,HF,WF]

    # ---- windowed cross-attention for the local stripe (gather-free) ----
    kr0 = idx * KL
    qb = q.reshape(B, HEADS, DH, KL, R, WF, R)                 # b n d i u j v

    # zero-pad key/value maps by HALF on each spatial side; invalid window
    # positions are masked to -1e9 before softmax (-> exactly 0 weight in f32)
    kpad = jnp.pad(kk, ((0, 0), (0, 0), (HALF, HALF), (HALF, HALF)))
    vpad = jnp.pad(features, ((0, 0), (0, 0), (HALF, HALF), (HALF, HALF)))
    # local halo window: rows kr0-HALF .. kr0+KL+HALF in padded coords = kr0..
    khalo = lax.dynamic_slice(kpad, (0, 0, kr0, 0), (B, QK, KL + 2 * HALF, WF + 2 * HALF))
    vhalo = lax.dynamic_slice(vpad, (0, 0, kr0, 0), (B, CV, KL + 2 * HALF, WF + 2 * HALF))
    khalo = khalo.reshape(B, HEADS, DH, KL + 2 * HALF, WF + 2 * HALF)
    vhalo = vhalo.reshape(B, HEADS, DV, KL + 2 * HALF, WF + 2 * HALF)

    il = kr0 + jnp.arange(KL)                                  # absolute key rows
    jl = jnp.arange(WF)
    dy = jnp.arange(-HALF, HALF + 1)
    rvalid = (il[:, None] + dy[None, :] >= 0) & (il[:, None] + dy[None, :] < HF)
    cvalid = (jl[:, None] + dy[None, :] >= 0) & (jl[:, None] + dy[None, :] < WF)
    # mask[i, j, p] with p = oy*WIN+ox
    wmask = (rvalid[:, None, :, None] & cvalid[None, :, None, :]).reshape(KL, WF, WIN * WIN)

    ksh = jnp.stack([khalo[:, :, :, oy:oy + KL, ox:ox + WF]
                     for oy in range(WIN) for ox in range(WIN)], axis=0)  # [P,b,n,d,i,j]
    vsh = jnp.stack([vhalo[:, :, :, oy:oy + KL, ox:ox + WF]
                     for oy in range(WIN) for ox in range(WIN)], axis=0)  # [P,b,n,e,i,j]

    scale = np.float32(1.0 / np.sqrt(DH))
    scores = jnp.einsum('bndiujv,pbndij->bniujvp', qb, ksh) * scale
    scores = jnp.where(wmask[None, None, :, None, :, None, :], scores, -1e9)
    m = scores.max(axis=-1, keepdims=True)
    e = jnp.exp(scores - m)
    attn = e * (1.0 / jnp.sum(e, axis=-1, keepdims=True))
    out = jnp.einsum('bniujvp,pbneij->bneiujv', attn, vsh)     # [B,N,DV,KL,R,WF,R]

    out = out.reshape(B, CV, KL, R, WF, R).reshape(B, CV, HL, W)
    return out


_fwd = None


def _get_fwd():
    global _fwd
    if _fwd is None:
        _fwd = jax.pmap(
            _fwd_stripe, axis_name='i',
            in_axes=(0,) + (None,) * 10,
            out_axes=0,
        )
    return _fwd


def kernel(**inputs):
    image = np.ascontiguousarray(np.asarray(inputs['image'], dtype=np.float32))
    img_s = image.reshape(B, 3, NC, HL, W).transpose(2, 0, 1, 3, 4)
    args = [np.asarray(inputs[k], dtype=np.float32) for k in
            ('features', 'pre_img', 'pre_key', 'pre_query', 'lfu_w', 'pre_agg',
             'rb_w1', 'rb_w2', 'rb_gamma', 'rb_beta')]
    out = _get_fwd()(img_s, *args)                      # [NC,B,CV,HL,W]
    out = np.asarray(out).transpose(1, 2, 0, 3, 4).reshape(B, CV, H, W)
    return out


# revision 16
# speedup vs baseline: 621.7610x; 3.4711x over previous
"""AnyUp sparse-attention upsampler on 8 Trainium2 NeuronCores.

Sharding: the full-res (256x256) encoder/query/key branches are sharded
row-wise into 8 stripes of 32 rows (pure spatial data parallel). GroupNorm
needs global spatial stats -> lax.psum of per-stripe partial sums. The
low-res (32x32) branch is cheap and computed replicated on every core;
the pooled key image (computed from sharded full-res rows) is all_gathered
so every core holds the full 32x32 key/value maps. Windowed cross-attention
is then computed locally per stripe (each stripe's 4 key-cell rows read a
+-2 row halo from the replicated key map). Output stripes are gathered on
host into the full [2,384,256,256] tensor.
"""

import numpy as np
import jax
import jax.numpy as jnp
from jax import lax
from functools import partial

QK = 128
HEADS = 4
GROUPS = 8
EPS = 1e-5
B = 2
CV = 384
H = W = 256
HF = WF = 32
NC = 8            # cores
HL = H // NC      # 32 local query rows per core
KL = HF // NC     # 4 local key-cell rows per core
R = H // HF       # 8 upsampling ratio
DH = QK // HEADS  # 32
DV = CV // HEADS  # 96
HALF = 2          # max(1, round(0.1*32/2))
WIN = 2 * HALF + 1


def _conv1x1(x, w):
    return jnp.einsum('bchw,oc->bohw', x, w)


def _gn_global(x, gamma, beta):
    # x: [B,C,HL,W] stripe; stats over the full H x W map via psum.
    b, c, h, w = x.shape
    xg = x.reshape(b, GROUPS, c // GROUPS, h, w)
    s1 = lax.psum(xg.sum(axis=(2, 3, 4)), 'i')
    s2 = lax.psum((xg * xg).sum(axis=(2, 3, 4)), 'i')
    n = (c // GROUPS) * H * w
    m = s1 / n
    v = s2 / n - m * m
    xn = (xg - m[..., None, None, None]) * lax.rsqrt(v[..., None, None, None] + EPS)
    xn = xn.reshape(b, c, h, w)
    return xn * gamma[None, :, None, None] + beta[None, :, None, None]


def _gn_local(x, gamma, beta):
    # full map held locally (low-res branch) — matches reference exactly
    b, c, h, w = x.shape
    xg = x.reshape(b, GROUPS, c // GROUPS, h, w)
    m = xg.mean(axis=(2, 3, 4), keepdims=True)
    v = xg.var(axis=(2, 3, 4), keepdims=True)
    xn = ((xg - m) / jnp.sqrt(v + EPS)).reshape(b, c, h, w)
    return xn * gamma[None, :, None, None] + beta[None, :, None, None]


def _conv_reflect(x, w):
    # k x k conv with reflect padding, as shifted-slice einsums
    # (conv_general_dilated ICEs this neuronxcc build)
    k = w.shape[-1]
    p = k // 2
    h, ww = x.shape[-2:]
    xp = jnp.pad(x, ((0, 0), (0, 0), (p, p), (p, p)), mode='reflect')
    out = None
    for dy in range(k):
        for dx in range(k):
            t = jnp.einsum('bchw,oc->bohw', xp[:, :, dy:dy + h, dx:dx + ww], w[:, :, dy, dx])
            out = t if out is None else out + t
    return out


def _rope_stripe(e, row0):
    # e: [B,C,HL,W]; axial 2D RoPE with absolute row coordinates row0..row0+HL
    b, c, h, w = e.shape
    d4 = c // 4
    freqs = 100.0 ** (jnp.arange(d4, dtype=jnp.float32) / d4)
    ys = (row0 + jnp.arange(h, dtype=jnp.float32) + 0.5) / H
    xs = (jnp.arange(w, dtype=jnp.float32) + 0.5) / W
    ay = ys[:, None] * freqs[None, :] * jnp.pi
    ax = xs[:, None] * freqs[None, :] * jnp.pi
    eh = e.transpose(0, 2, 3, 1)
    vy = eh[..., :c // 2].reshape(b, h, w, d4, 2)
    vx = eh[..., c // 2:].reshape(b, h, w, d4, 2)

    def rot(v, cc, ss):
        return jnp.stack([v[..., 0] * cc - v[..., 1] * ss,
                          v[..., 0] * ss + v[..., 1] * cc], axis=-1)

    vy = rot(vy, jnp.cos(ay)[None, :, None, :], jnp.sin(ay)[None, :, None, :])
    vx = rot(vx, jnp.cos(ax)[None, None, :, :], jnp.sin(ax)[None, None, :, :])
    out = jnp.concatenate([vy.reshape(b, h, w, c // 2),
                           vx.reshape(b, h, w, c // 2)], axis=-1)
    return out.transpose(0, 3, 1, 2)


def _fwd_stripe(image_s, features, pre_img, pre_key, pre_query, lfu_w, pre_agg,
                rb_w1, rb_w2, rb_gamma, rb_beta):
    idx = lax.axis_index('i')
    row0 = (idx * HL).astype(jnp.float32)

    def rb_g(x, i):
        h = _conv1x1(x, rb_w1[i])
        h = _gn_global(h, rb_gamma[i], rb_beta[i])
        h = jax.nn.silu(h)
        h = _conv1x1(h, rb_w2[i])
        return x + h

    def rb_l(x, i):
        h = _conv1x1(x, rb_w1[i])
        h = _gn_local(h, rb_gamma[i], rb_beta[i])
        h = jax.nn.silu(h)
        h = _conv1x1(h, rb_w2[i])
        return x + h

    # ---- full-res trunk (sharded by rows) ----
    enc = rb_g(rb_g(_conv1x1(image_s, pre_img), 0), 1)
    enc = _rope_stripe(enc, row0)

    q = rb_g(rb_g(_conv1x1(enc, pre_query), 4), 5)            # [B,QK,HL,W]
    k_pre = rb_g(rb_g(_conv1x1(enc, pre_key), 2), 3)          # [B,QK,HL,W]
    k_img_loc = k_pre.reshape(B, QK, KL, R, WF, R).mean(axis=(3, 5))  # [B,QK,KL,WF]

    # ---- low-res branch (replicated) ----
    fn = features / jnp.maximum(jnp.linalg.norm(features, axis=1, keepdims=True), 1e-12)
    kfe = _conv_reflect(fn.mean(axis=1, keepdims=True), lfu_w)
    kfe = rb_l(rb_l(kfe, 6), 7)                                # [B,QK,HF,WF]

    # gather the pooled key image across cores -> full 32x32 map
    k_img = lax.all_gather(k_img_loc, 'i')                     # [NC,B,QK,KL,WF]
    k_img = k_img.transpose(1, 2, 0, 3, 4).reshape(B, QK, HF, WF)

    kk = jnp.concatenate([k_img, kfe], axis=1)
    kk = rb_l(rb_l(_conv_reflect(kk, pre_agg), 8), 9)          # [B,QK,HF,WF]

    # ---- windowed cross-attention for the local stripe (gather-free) ----
    kr0 = idx * KL
    qb = q.reshape(B, HEADS, DH, KL, R, WF, R)                 # b n d i u j v
    del q

    # zero-pad key/value maps by HALF on each spatial side; invalid window
    # positions are masked to -1e9 before softmax (-> exactly 0 weight in f32)
    # attention tail in bf16 (f32 accumulation); trunk stays f32 so the
    # quantization error does not compound through the resblock chain
    qb = qb.astype(jnp.bfloat16)
    kpad = jnp.pad(kk.astype(jnp.bfloat16), ((0, 0), (0, 0), (HALF, HALF), (HALF, HALF)))
    vpad = jnp.pad(features.astype(jnp.bfloat16), ((0, 0), (0, 0), (HALF, HALF), (HALF, HALF)))
    # local halo window: rows kr0-HALF .. kr0+KL+HALF in padded coords = kr0..
    khalo = lax.dynamic_slice(kpad, (0, 0, kr0, 0), (B, QK, KL + 2 * HALF, WF + 2 * HALF))
    vhalo = lax.dynamic_slice(vpad, (0, 0, kr0, 0), (B, CV, KL + 2 * HALF, WF + 2 * HALF))
    khalo = khalo.reshape(B, HEADS, DH, KL + 2 * HALF, WF + 2 * HALF)
    vhalo = vhalo.reshape(B, HEADS, DV, KL + 2 * HALF, WF + 2 * HALF)

    il = kr0 + jnp.arange(KL)                                  # absolute key rows
    jl = jnp.arange(WF)
    dy = jnp.arange(-HALF, HALF + 1)
    rvalid = (il[:, None] + dy[None, :] >= 0) & (il[:, None] + dy[None, :] < HF)
    cvalid = (jl[:, None] + dy[None, :] >= 0) & (jl[:, None] + dy[None, :] < WF)
    # mask[i, j, p] with p = oy*WIN+ox
    wmask = (rvalid[:, None, :, None] & cvalid[None, :, None, :]).reshape(KL, WF, WIN * WIN)

    ksh = jnp.stack([khalo[:, :, :, oy:oy + KL, ox:ox + WF]
                     for oy in range(WIN) for ox in range(WIN)], axis=0)  # [P,b,n,d,i,j]
    vsh = jnp.stack([vhalo[:, :, :, oy:oy + KL, ox:ox + WF]
                     for oy in range(WIN) for ox in range(WIN)], axis=0)  # [P,b,n,e,i,j]

    scale = np.float32(1.0 / np.sqrt(DH))
    scores = jnp.einsum('bndiujv,pbndij->bniujvp', qb, ksh,
                        preferred_element_type=jnp.float32) * scale
    scores = jnp.where(wmask[None, None, :, None, :, None, :], scores, -1e9)
    attn = jax.nn.softmax(scores, axis=-1)
    out = jnp.einsum('bniujvp,pbneij->bneiujv', attn.astype(jnp.bfloat16), vsh,
                     preferred_element_type=jnp.float32)       # [B,N,DV,KL,R,WF,R]

    out = out.reshape(B, CV, KL, R, WF, R).reshape(B, CV, HL, W)
    return out.astype(jnp.float32)


_fwd = None


def _get_fwd():
    global _fwd
    if _fwd is None:
        _fwd = jax.pmap(
            _fwd_stripe, axis_name='i',
            in_axes=(0,) + (None,) * 10,
            out_axes=0,
        )
    return _fwd


def kernel(**inputs):
    image = np.ascontiguousarray(np.asarray(inputs['image'], dtype=np.float32))
    img_s = image.reshape(B, 3, NC, HL, W).transpose(2, 0, 1, 3, 4)
    args = [np.asarray(inputs[k], dtype=np.float32) for k in
            ('features', 'pre_img', 'pre_key', 'pre_query', 'lfu_w', 'pre_agg',
             'rb_w1', 'rb_w2', 'rb_gamma', 'rb_beta')]
    out = _get_fwd()(img_s, *args)                      # [NC,B,CV,HL,W]
    out = np.asarray(out).transpose(1, 2, 0, 3, 4).reshape(B, CV, H, W)
    return out
